# revision 1
# baseline (speedup 1.0000x reference)
"""Trainium2 Bass kernel for nn_ModelWithAuxiliaryHead (moe_routing).

Strategy
--------
The only heavy compute is the lm-head: logits = tokens @ W_lm for
2560 tokens (2048 last_hidden + 512 LoRA-routed math tokens) against
W_lm [4096, 32000], followed by a per-token logsumexp.  That is
671 GFLOP and dominates everything else by >100x.

Device (8 NeuronCores, vocab-parallel):
  - Each core owns a 4000-column vocab shard of W_lm (8 chunks of 500).
  - Tokens (bf16, transposed) are replicated; W shard is bf16.
  - For each (token-tile of 128) x (vocab chunk of 500): accumulate
    32 K-step matmuls into PSUM (fp32), then one ScalarE Exp with
    accum_out produces the per-token sum-of-exp for that chunk.
    Logits are bounded (|logit| < ~20), so no max-subtraction is needed.
  - Output per core: [128, n_token_tiles * n_chunks] fp32 partial sumexp.

Host (cheap, exact fp32):
  - math-token gather + segment-routed LoRA (A @ (B @ h) + bias): 2 GFLOP.
  - label logits   logit[t, label[t]] = tokens[t] . W[:, label[t]].
  - combine: lse = log(sum of 8 cores' sumexp); nll = lse - label_logit;
    masked means / per-segment means -> the 5 outputs.
"""

import sys

if "/opt/trn_rl_repo" not in sys.path:
    sys.path.insert(0, "/opt/trn_rl_repo")

import numpy as np
import ml_dtypes

BF16 = ml_dtypes.bfloat16

# problem shapes (hardcoded per spec)
B, S, H, V, Sm = 2, 1024, 4096, 32000, 256
NSEG, CHUNK = 16, 16
BETA1, BETA2, BETA3 = 0.5, 0.5, 0.4
N_CORES = 8
TOK = B * S + B * Sm            # 2560 tokens through the lm head
VS = V // N_CORES               # 4000 vocab columns per core

# device tiling
P = 128                         # partitions / token-tile size
CW = 500                        # vocab chunk width (<=512 fp32 PSUM bank)
NCC = VS // CW                  # 8 vocab chunks per core
KT = H // P                     # 32 K-tiles
N_PASS = 2                      # token passes (SBUF residency)
TT = TOK // (N_PASS * P)        # 10 token-tiles per pass

_cache = {}


def _build_nc(n_pass, tt, kt, ncc, cw, hid_bufs, w_bufs):
    """Build + compile the SPMD Bass kernel. Token count = n_pass*tt*128,
    hidden = kt*128, vocab shard = ncc*cw."""
    import concourse.mybir as mybir
    import concourse.tile as tile
    from concourse import bacc

    nc = bacc.Bacc("TRN2", target_bir_lowering=False, debug=False,
                   num_devices=N_CORES)
    bf = mybir.dt.bfloat16
    f32 = mybir.dt.float32

    hid_d = nc.dram_tensor("hid", [n_pass, kt, P, tt * P], bf,
                           kind="ExternalInput")
    w_d = nc.dram_tensor("w", [ncc, kt, P, cw], bf, kind="ExternalInput")
    se_d = nc.dram_tensor("se", [P, n_pass * tt * ncc], f32,
                          kind="ExternalOutput")

    with tile.TileContext(nc) as tc:
        with (
            tc.tile_pool(name="hidp", bufs=hid_bufs) as hidp,
            tc.tile_pool(name="wp", bufs=w_bufs) as wp,
            tc.tile_pool(name="sep", bufs=1) as sep,
            tc.tile_pool(name="exp", bufs=4) as expp,
            tc.tile_pool(name="ps", bufs=8, space="PSUM") as psp,
        ):
            se_all = sep.tile([P, n_pass * tt * ncc], f32)
            for pas in range(n_pass):
                hid_t = []
                for k in range(kt):
                    t = hidp.tile([P, tt * P], bf, tag="hid")
                    nc.sync.dma_start(t[:], hid_d[pas, k])
                    hid_t.append(t)
                for cc in range(ncc):
                    w_t = []
                    for k in range(kt):
                        t = wp.tile([P, cw], bf, tag="w")
                        nc.sync.dma_start(t[:], w_d[cc, k])
                        w_t.append(t)
                    for ti in range(tt):
                        ps = psp.tile([P, cw], f32, tag="ps")
                        for k in range(kt):
                            nc.tensor.matmul(
                                ps[:],
                                hid_t[k][:, ti * P:(ti + 1) * P],
                                w_t[k][:],
                                start=(k == 0),
                                stop=(k == kt - 1),
                            )
                        ex = expp.tile([P, cw], f32, tag="ex")
                        col = (pas * tt + ti) * ncc + cc
                        nc.scalar.activation(
                            ex[:], ps[:],
                            mybir.ActivationFunctionType.Exp,
                            accum_out=se_all[:, col:col + 1],
                        )
            nc.sync.dma_start(se_d[:], se_all[:])
    nc.compile()
    return nc


def _get_nc():
    key = "full"
    if key not in _cache:
        _cache[key] = _build_nc(N_PASS, TT, KT, NCC, CW, hid_bufs=48,
                                w_bufs=48)
    return _cache[key]


def _pack_inputs(tokens_f32, W_lm):
    """tokens [TOK, H] fp32, W_lm [H, V] fp32 -> per-core in_maps."""
    # hid: [n_pass, kt, 128, tt*128]; hid[p, k, i, j] = tokens[p*1280 + j, k*128 + i]
    hidT = np.ascontiguousarray(tokens_f32.T.astype(BF16))        # [H, TOK]
    hid = hidT.reshape(KT, P, N_PASS, TT * P).transpose(2, 0, 1, 3)
    hid = np.ascontiguousarray(hid)
    W_b = W_lm.astype(BF16)                                        # [H, V]
    in_maps = []
    for c in range(N_CORES):
        ws = W_b[:, c * VS:(c + 1) * VS]                           # [H, 4000]
        w = ws.reshape(KT, P, NCC, CW).transpose(2, 0, 1, 3)
        in_maps.append({"hid": hid, "w": np.ascontiguousarray(w)})
    return in_maps


def _run_device(tokens_f32, W_lm):
    """Returns sumexp [TOK] fp32 (full-vocab sum of exp(logits))."""
    from concourse.bass_utils import run_bass_kernel_spmd
    nc = _get_nc()
    in_maps = _pack_inputs(tokens_f32, W_lm)
    res = run_bass_kernel_spmd(nc, in_maps, core_ids=list(range(N_CORES)))
    # se [128, n_pass*tt*ncc]; token t = tcol*128 + p, tcol = col // ncc
    total = np.zeros(TOK, np.float64)
    for c in range(N_CORES):
        se = res.results[c]["se"].astype(np.float64)               # [128, 160]
        se = se.reshape(P, N_PASS * TT, NCC).sum(axis=2)           # [128, 20]
        total += se.T.reshape(-1)                                  # token-major
    return total


def _host_prep(inputs):
    """LoRA math tokens + token concat, all fp32 exact."""
    kh = np.asarray(inputs["k_hidden"], np.float32)
    lh = np.asarray(inputs["last_hidden"], np.float32)
    starts = np.asarray(inputs["starts"], np.int64)
    A = np.asarray(inputs["A_matrices"], np.float32)
    Bm = np.asarray(inputs["B_matrices"], np.float32)
    bias = np.asarray(inputs["bias"], np.float32)

    pos = starts[:, None] + np.arange(Sm)[None, :]                 # [B, Sm]
    math_h = np.take_along_axis(kh, pos[:, :, None], axis=1)       # [B, Sm, H]
    hb = math_h.reshape(B, NSEG, CHUNK, H)
    # grouped GEMM: [B*CHUNK, H] @ [H, R] per segment
    inter = np.einsum("bsch,srh->bscr", hb, Bm, optimize=True)
    trans = np.einsum("bscr,shr->bsch", inter, A, optimize=True)
    trans = trans + bias[None, :, None, :]
    t_math = np.ascontiguousarray(trans.reshape(B * Sm, H), dtype=np.float32)
    tokens = np.concatenate([lh.reshape(B * S, H), t_math], axis=0)
    return tokens


def kernel(**inputs):
    tokens = _host_prep(inputs)
    W = np.asarray(inputs["W_lm"], np.float32)
    input_ids = np.asarray(inputs["input_ids"], np.int64)
    attention_mask = np.asarray(inputs["attention_mask"], np.int64)
    starts = np.asarray(inputs["starts"], np.int64)
    ends = np.asarray(inputs["ends"], np.int64)
    mlabels = np.asarray(inputs["math_labels"], np.int64)

    sumexp = _run_device(tokens, W)                                # [2560]
    lse = np.log(sumexp)                                           # float64

    # exact label logits on host
    labels = input_ids[:, 1:]                                      # [B, S-1]
    Wg = W[:, labels.reshape(-1)]                                  # [H, B*(S-1)]
    tok_bs = tokens[:B * S].reshape(B, S, H)[:, :-1, :].reshape(-1, H)
    ll_simple = np.einsum("th,ht->t", tok_bs, Wg,
                          optimize=True).reshape(B, S - 1)
    Wm = W[:, mlabels.reshape(-1)]                                 # [H, B*Sm]
    ll_math = np.einsum("th,ht->t", tokens[B * S:], Wm, optimize=True)

    lse_bs = lse[:B * S].reshape(B, S)[:, :-1]
    nll = (lse_bs - ll_simple).astype(np.float32)                  # [B, S-1]
    mnll = (lse[B * S:] - ll_math).astype(np.float32)              # [B*Sm]

    idx = np.arange(S - 1)[None, :]
    real_len = attention_mask.sum(axis=1)[:, None]
    mask_simple = (idx >= starts[:, None] - 1) & (idx <= ends[:, None] - 1)
    mask_final = (idx >= ends[:, None]) & (idx < real_len - 1)

    def masked_mean(x, m):
        m = m.astype(np.float32)
        s = m.sum()
        if s > 0:
            return np.float32((x * m).sum() / max(s, np.float32(1.0)))
        return np.float32(0.0)

    simple_talk_loss = masked_mean(nll, mask_simple)
    final_answer_loss = masked_mean(nll, mask_final)
    math_loss = np.float32(mnll.mean())
    A_losses = mnll.reshape(B, NSEG, CHUNK).mean(axis=(0, 2)).astype(np.float32)
    total_loss = np.float32(BETA1 * math_loss + BETA2 * simple_talk_loss
                            + BETA3 * final_answer_loss)
    return (np.asarray(total_loss, np.float32),
            np.asarray(math_loss, np.float32),
            np.asarray(simple_talk_loss, np.float32),
            np.asarray(final_answer_loss, np.float32),
            A_losses)


# revision 18
# speedup vs baseline: 9869.1098x; 9869.1098x over previous
"""Trainium2 Bass kernel for nn_ModelWithAuxiliaryHead (moe_routing).

Strategy
--------
The only heavy compute is the lm-head: logits = tokens @ W_lm for
2560 tokens (2048 last_hidden + 512 LoRA-routed math tokens) against
W_lm [4096, 32000], followed by a per-token logsumexp.  That is
671 GFLOP and dominates everything else by >100x.

Device (8 NeuronCores, vocab-parallel):
  - Each core owns a 4000-column vocab shard of W_lm (8 chunks of 500).
  - Tokens (bf16, transposed) are replicated; W shard is bf16.
  - For each (token-tile of 128) x (vocab chunk of 500): accumulate
    32 K-step matmuls into PSUM (fp32), then one ScalarE Exp with
    accum_out produces the per-token sum-of-exp for that chunk.
    Logits are bounded (|logit| < ~20), so no max-subtraction is needed.
  - Output per core: [128, n_token_tiles * n_chunks] fp32 partial sumexp.

Host (cheap, exact fp32):
  - math-token gather + segment-routed LoRA (A @ (B @ h) + bias): 2 GFLOP.
  - label logits   logit[t, label[t]] = tokens[t] . W[:, label[t]].
  - combine: lse = log(sum of 8 cores' sumexp); nll = lse - label_logit;
    masked means / per-segment means -> the 5 outputs.
"""

import sys

if "/opt/trn_rl_repo" not in sys.path:
    sys.path.insert(0, "/opt/trn_rl_repo")

import numpy as np
import ml_dtypes

BF16 = ml_dtypes.bfloat16

# problem shapes (hardcoded per spec)
B, S, H, V, Sm = 2, 1024, 4096, 32000, 256
NSEG, CHUNK = 16, 16
BETA1, BETA2, BETA3 = 0.5, 0.5, 0.4
N_CORES = 8
TOK = B * S + B * Sm            # 2560 tokens through the lm head
VS = V // N_CORES               # 4000 vocab columns per core

# device tiling
P = 128                         # partitions / token-tile size
CW = 500                        # vocab chunk width (<=512 fp32 PSUM bank)
NCC = VS // CW                  # 8 vocab chunks per core
KT = H // P                     # 32 K-tiles
N_PASS = 2                      # token passes (SBUF residency)
TT = TOK // (N_PASS * P)        # 10 token-tiles per pass

_cache = {}

# fp8 mode constants
F8 = ml_dtypes.float8_e4m3fn
W_SCALE = 64.0
KT2 = H // 256                  # 16 K-supertiles (256 rows each, DoubleRow)
CW8 = 512                       # fp8 vocab chunk width
VS8 = NCC * CW8                 # 4096 padded shard width (96 zero pads)
TT8 = TOK // P                  # 20 token tiles, single pass


def _build_nc_fp8(tt, kt2, ncc, cw, repeat=1, repeat_compute=1):
    """fp8-e4m3 DoubleRow variant: single pass, K=256 per matmul.

    Token count = tt*128, hidden = kt2*256, vocab shard = ncc*cw (incl
    zero padding). W is pre-scaled by 64 on the host; the Exp activation
    applies scale=1/64. Each pad column contributes exp(0)=1 to the
    sumexp; the host subtracts the pad count.
    """
    import concourse.mybir as mybir
    import concourse.tile as tile
    from concourse import bacc

    nc = bacc.Bacc("TRN2", target_bir_lowering=False, debug=False,
                   num_devices=N_CORES)
    f8 = mybir.dt.float8e4
    f32 = mybir.dt.float32

    KG = 4 if kt2 % 4 == 0 else kt2   # k2-super-tiles per hid DMA
    n_hg = kt2 // KG
    hid_d = nc.dram_tensor("hid", [n_hg, P, KG * 2 * tt * P], f8,
                           kind="ExternalInput")
    w_d = nc.dram_tensor("w", [ncc, P, kt2 * 2 * cw], f8,
                         kind="ExternalInput")
    se_d = nc.dram_tensor("se", [P, tt * ncc], f32, kind="ExternalOutput")

    with tile.TileContext(nc) as tc:
        with (
            tc.tile_pool(name="hidp", bufs=n_hg) as hidp,
            tc.tile_pool(name="wp", bufs=2) as wp,
            tc.tile_pool(name="sep", bufs=1) as sep,
            tc.tile_pool(name="exp", bufs=4) as expp,
            tc.tile_pool(name="ps", bufs=8, space="PSUM") as psp,
        ):
            se_all = sep.tile([P, tt * ncc], f32)
            for _rep in range(repeat):
                hid_t = []               # hid_t[g] : [P, KG, 2, tt*P]
                for g in range(n_hg):
                    t = hidp.tile([P, KG, 2, tt * P], f8, tag="hid")
                    nc.sync.dma_start(t[:], hid_d[g])
                    hid_t.append(t)
                for cc in range(ncc):
                    w_t = wp.tile([P, kt2, 2, cw], f8, tag="w")
                    nc.sync.dma_start(w_t[:], w_d[cc])
                    for _crep in range(repeat_compute):
                      for ti in range(tt):
                        ps = psp.tile([P, cw], f32, tag="ps")
                        for k in range(kt2):
                            nc.tensor.matmul(
                                ps[:],
                                hid_t[k // KG][:, k % KG, :,
                                               ti * P:(ti + 1) * P],
                                w_t[:, k, :, :],
                                start=(k == 0),
                                stop=(k == kt2 - 1),
                                perf_mode=mybir.MatmulPerfMode.DoubleRow,
                            )
                        ex = expp.tile([P, cw], f32, tag="ex")
                        col = ti * ncc + cc
                        nc.scalar.activation(
                            ex[:], ps[:],
                            mybir.ActivationFunctionType.Exp,
                            scale=1.0 / W_SCALE,
                            accum_out=se_all[:, col:col + 1],
                        )
            nc.sync.dma_start(se_d[:], se_all[:])
    nc.compile()
    return nc


def _build_nc(n_pass, tt, kt, ncc, cw, hid_bufs, w_bufs, repeat=1,
              repeat_compute=1):
    """Build + compile the SPMD Bass kernel. Token count = n_pass*tt*128,
    hidden = kt*128, vocab shard = ncc*cw. repeat>1 duplicates the whole
    compute; repeat_compute>1 duplicates only the matmul+exp stage
    (timing-slope measurements only)."""
    import concourse.mybir as mybir
    import concourse.tile as tile
    from concourse import bacc

    nc = bacc.Bacc("TRN2", target_bir_lowering=False, debug=False,
                   num_devices=N_CORES)
    bf = mybir.dt.bfloat16
    f32 = mybir.dt.float32

    KG = 8 if kt % 8 == 0 else kt   # hid DMA granularity (k-tiles per DMA)
    n_hg = kt // KG
    hid_d = nc.dram_tensor("hid", [n_pass, n_hg, P, KG * tt * P], bf,
                           kind="ExternalInput")
    w_d = nc.dram_tensor("w", [ncc, P, kt * cw], bf, kind="ExternalInput")
    se_d = nc.dram_tensor("se", [P, n_pass * tt * ncc], f32,
                          kind="ExternalOutput")

    with tile.TileContext(nc) as tc:
        with (
            tc.tile_pool(name="hidp", bufs=hid_bufs) as hidp,
            tc.tile_pool(name="wp", bufs=w_bufs) as wp,
            tc.tile_pool(name="sep", bufs=1) as sep,
            tc.tile_pool(name="exp", bufs=4) as expp,
            tc.tile_pool(name="ps", bufs=8, space="PSUM") as psp,
        ):
            se_all = sep.tile([P, n_pass * tt * ncc], f32)
            for _rep in range(repeat):
              for pas in range(n_pass):
                hid_t = []                       # hid_t[g] : [P, KG, tt*P]
                for g in range(n_hg):
                    t = hidp.tile([P, KG, tt * P], bf, tag="hid")
                    nc.sync.dma_start(t[:], hid_d[pas, g])
                    hid_t.append(t)
                for cc in range(ncc):
                    w_t = wp.tile([P, kt, cw], bf, tag="w")
                    nc.sync.dma_start(w_t[:], w_d[cc])
                    for _crep in range(repeat_compute):
                      for ti in range(tt):
                        ps = psp.tile([P, cw], f32, tag="ps")
                        for k in range(kt):
                            nc.tensor.matmul(
                                ps[:],
                                hid_t[k // KG][:, k % KG,
                                               ti * P:(ti + 1) * P],
                                w_t[:, k, :],
                                start=(k == 0),
                                stop=(k == kt - 1),
                            )
                        ex = expp.tile([P, cw], f32, tag="ex")
                        col = (pas * tt + ti) * ncc + cc
                        nc.scalar.activation(
                            ex[:], ps[:],
                            mybir.ActivationFunctionType.Exp,
                            accum_out=se_all[:, col:col + 1],
                        )
            nc.sync.dma_start(se_d[:], se_all[:])
    nc.compile()
    return nc


MODE = "fp8"                    # "fp8" (DoubleRow) or "bf16"


def _get_nc():
    key = "full_" + MODE
    if key not in _cache:
        if MODE == "fp8":
            _cache[key] = _build_nc_fp8(TT8, KT2, NCC, CW8)
        else:
            _cache[key] = _build_nc(N_PASS, TT, KT, NCC, CW, hid_bufs=5,
                                    w_bufs=2)
    return _cache[key]


def _pack_inputs_fp8(tokens_f32, W_lm):
    """tokens [TOK, H] fp32, W_lm [H, V] fp32 -> per-core fp8 in_maps."""
    KG = 4
    n_hg = KT2 // KG
    # hid[g, p, (k2g, j, t)] = tokens[t, ((g*KG + k2g)*2 + j)*128 + p]
    hidT = tokens_f32.T.astype(F8)                                 # [H, TOK]
    hid = hidT.reshape(n_hg, KG, 2, P, TOK).transpose(0, 3, 1, 2, 4)
    hid = np.ascontiguousarray(hid).reshape(n_hg, P, KG * 2 * TOK)
    Ws = (W_lm * W_SCALE).astype(F8)                               # [H, V]
    in_maps = []
    for c in range(N_CORES):
        ws = Ws[:, c * VS:(c + 1) * VS]                            # [H, 4000]
        wp = np.zeros((H, VS8), F8)
        wp[:, :VS] = ws
        # w[cc, p, (k2, j, n)] = wp[(k2*2 + j)*128 + p, cc*cw + n]
        w = wp.reshape(KT2, 2, P, NCC, CW8).transpose(3, 2, 0, 1, 4)
        w = np.ascontiguousarray(w).reshape(NCC, P, KT2 * 2 * CW8)
        in_maps.append({"hid": hid, "w": w})
    return in_maps


def _pack_inputs(tokens_f32, W_lm):
    """tokens [TOK, H] fp32, W_lm [H, V] fp32 -> per-core in_maps."""
    KG = 8
    n_hg = KT // KG
    # hid[pas, g, p, (k8, t)] = tokens[pas*tt*128 + t, (g*KG + k8)*128 + p]
    hidT = tokens_f32.T.astype(BF16)                               # [H, TOK]
    hid = hidT.reshape(n_hg, KG, P, N_PASS, TT * P).transpose(3, 0, 2, 1, 4)
    hid = np.ascontiguousarray(hid).reshape(N_PASS, n_hg, P, KG * TT * P)
    W_b = W_lm.astype(BF16)                                        # [H, V]
    in_maps = []
    for c in range(N_CORES):
        ws = W_b[:, c * VS:(c + 1) * VS]                           # [H, 4000]
        # w[cc, p, (k, n)] = ws[k*128 + p, cc*cw + n]
        w = ws.reshape(KT, P, NCC, CW).transpose(2, 1, 0, 3)
        w = np.ascontiguousarray(w).reshape(NCC, P, KT * CW)
        in_maps.append({"hid": hid, "w": w})
    return in_maps


def _make_runner(nc):
    """Build a jitted SPMD executable for an already-compiled nc.

    Returns (fn, mesh, out_avals) where fn(hid_global, w_global,
    zero_se_global) -> [se_global]. hid is replicated; w/se are sharded
    on axis 0.  (Used by test.py for repeat-timing; kernel() itself goes
    through run_bass_kernel_spmd.)
    """
    import jax
    import concourse.mybir as mybir
    from concourse.bass2jax import (_bass_exec_p, install_neuronx_cc_hook,
                                    partition_id_tensor)
    from jax.experimental.shard_map import shard_map
    from jax.sharding import Mesh, PartitionSpec

    install_neuronx_cc_hook()

    part_name = (nc.partition_id_tensor.name
                 if nc.partition_id_tensor is not None else None)
    in_names, out_names, out_avals = [], [], []
    for alloc in nc.m.functions[0].allocations:
        if not isinstance(alloc, mybir.MemoryLocationSet):
            continue
        name = alloc.memorylocations[0].name
        if alloc.kind == "ExternalInput":
            if name != part_name:
                in_names.append(name)
        elif alloc.kind == "ExternalOutput":
            out_names.append(name)
            out_avals.append(jax.core.ShapedArray(
                tuple(alloc.tensor_shape), mybir.dt.np(alloc.dtype)))
    n_params = len(in_names)
    all_names = in_names + out_names
    if part_name is not None:
        all_names = all_names + [part_name]

    def _body(*args):
        operands = list(args)
        if part_name is not None:
            operands.append(partition_id_tensor())
        outs = _bass_exec_p.bind(
            *operands,
            out_avals=tuple(out_avals),
            in_names=tuple(all_names),
            out_names=tuple(out_names),
            lowering_input_output_aliases=(),
            sim_require_finite=True,
            sim_require_nnan=True,
            nc=nc,
        )
        return tuple(outs)

    devices = jax.devices()[:N_CORES]
    mesh = Mesh(np.asarray(devices), ("core",))
    # hid replicated, w + zero-out sharded along axis 0.  No donation:
    # the kernel writes every element of its outputs, so the pre-zeroed
    # "output" operands can be reused across calls (repeat timing).
    in_specs = (PartitionSpec(), PartitionSpec("core"), PartitionSpec("core"))
    out_specs = (PartitionSpec("core"),) * len(out_names)
    del n_params
    fn = jax.jit(
        shard_map(_body, mesh=mesh, in_specs=in_specs, out_specs=out_specs,
                  check_rep=False),
        keep_unused=True)
    return fn, mesh, out_avals


def _get_runner():
    if "runner" not in _cache:
        _cache["runner"] = _make_runner(_get_nc())
    return _cache["runner"]


def _run_device(tokens_f32, W_lm):
    """Returns sumexp [TOK] fp32 (full-vocab sum of exp(logits))."""
    from concourse.bass_utils import run_bass_kernel_spmd
    nc = _get_nc()
    if MODE == "fp8":
        in_maps = _pack_inputs_fp8(tokens_f32, W_lm)
    else:
        in_maps = _pack_inputs(tokens_f32, W_lm)
    res = run_bass_kernel_spmd(nc, in_maps, core_ids=list(range(N_CORES)))
    # se [128, n_tiles*ncc]; token t = tcol*128 + p, tcol = col // ncc
    n_tiles = TOK // P
    total = np.zeros((P, n_tiles), np.float64)
    for c in range(N_CORES):
        se = res.results[c]["se"].astype(np.float64)
        total += se.reshape(P, n_tiles, NCC).sum(axis=2)           # [128, 20]
    if MODE == "fp8":
        # each core's shard has VS8-VS zero-pad columns contributing exp(0)=1
        total -= N_CORES * (VS8 - VS)
    return total.T.reshape(-1)                                     # token-major


def _host_prep(inputs):
    """LoRA math tokens + token concat, all fp32 exact."""
    kh = np.asarray(inputs["k_hidden"], np.float32)
    lh = np.asarray(inputs["last_hidden"], np.float32)
    starts = np.asarray(inputs["starts"], np.int64)
    A = np.asarray(inputs["A_matrices"], np.float32)
    Bm = np.asarray(inputs["B_matrices"], np.float32)
    bias = np.asarray(inputs["bias"], np.float32)

    pos = starts[:, None] + np.arange(Sm)[None, :]                 # [B, Sm]
    math_h = np.take_along_axis(kh, pos[:, :, None], axis=1)       # [B, Sm, H]
    hb = math_h.reshape(B, NSEG, CHUNK, H)
    # grouped GEMM: [B*CHUNK, H] @ [H, R] per segment
    inter = np.einsum("bsch,srh->bscr", hb, Bm, optimize=True)
    trans = np.einsum("bscr,shr->bsch", inter, A, optimize=True)
    trans = trans + bias[None, :, None, :]
    t_math = np.ascontiguousarray(trans.reshape(B * Sm, H), dtype=np.float32)
    tokens = np.concatenate([lh.reshape(B * S, H), t_math], axis=0)
    return tokens


def kernel(**inputs):
    tokens = _host_prep(inputs)
    W = np.asarray(inputs["W_lm"], np.float32)
    input_ids = np.asarray(inputs["input_ids"], np.int64)
    attention_mask = np.asarray(inputs["attention_mask"], np.int64)
    starts = np.asarray(inputs["starts"], np.int64)
    ends = np.asarray(inputs["ends"], np.int64)
    mlabels = np.asarray(inputs["math_labels"], np.int64)

    sumexp = _run_device(tokens, W)                                # [2560]
    lse = np.log(sumexp)                                           # float64

    # exact label logits on host
    labels = input_ids[:, 1:]                                      # [B, S-1]
    Wg = W[:, labels.reshape(-1)]                                  # [H, B*(S-1)]
    tok_bs = tokens[:B * S].reshape(B, S, H)[:, :-1, :].reshape(-1, H)
    ll_simple = np.einsum("th,ht->t", tok_bs, Wg,
                          optimize=True).reshape(B, S - 1)
    Wm = W[:, mlabels.reshape(-1)]                                 # [H, B*Sm]
    ll_math = np.einsum("th,ht->t", tokens[B * S:], Wm, optimize=True)

    lse_bs = lse[:B * S].reshape(B, S)[:, :-1]
    nll = (lse_bs - ll_simple).astype(np.float32)                  # [B, S-1]
    mnll = (lse[B * S:] - ll_math).astype(np.float32)              # [B*Sm]

    idx = np.arange(S - 1)[None, :]
    real_len = attention_mask.sum(axis=1)[:, None]
    mask_simple = (idx >= starts[:, None] - 1) & (idx <= ends[:, None] - 1)
    mask_final = (idx >= ends[:, None]) & (idx < real_len - 1)

    def masked_mean(x, m):
        m = m.astype(np.float32)
        s = m.sum()
        if s > 0:
            return np.float32((x * m).sum() / max(s, np.float32(1.0)))
        return np.float32(0.0)

    simple_talk_loss = masked_mean(nll, mask_simple)
    final_answer_loss = masked_mean(nll, mask_final)
    math_loss = np.float32(mnll.mean())
    A_losses = mnll.reshape(B, NSEG, CHUNK).mean(axis=(0, 2)).astype(np.float32)
    total_loss = np.float32(BETA1 * math_loss + BETA2 * simple_talk_loss
                            + BETA3 * final_answer_loss)
    return (np.asarray(total_loss, np.float32),
            np.asarray(math_loss, np.float32),
            np.asarray(simple_talk_loss, np.float32),
            np.asarray(final_answer_loss, np.float32),
            A_losses)


# revision 20
# speedup vs baseline: 10167.8806x; 1.0303x over previous
"""Trainium2 Bass kernel for nn_ModelWithAuxiliaryHead (moe_routing).

Strategy
--------
The only heavy compute is the lm-head: logits = tokens @ W_lm for
2560 tokens (2048 last_hidden + 512 LoRA-routed math tokens) against
W_lm [4096, 32000], followed by a per-token logsumexp.  That is
671 GFLOP and dominates everything else by >100x.

Device (8 NeuronCores, vocab-parallel):
  - Each core owns a 4000-column vocab shard of W_lm (8 chunks of 500).
  - Tokens (bf16, transposed) are replicated; W shard is bf16.
  - For each (token-tile of 128) x (vocab chunk of 500): accumulate
    32 K-step matmuls into PSUM (fp32), then one ScalarE Exp with
    accum_out produces the per-token sum-of-exp for that chunk.
    Logits are bounded (|logit| < ~20), so no max-subtraction is needed.
  - Output per core: [128, n_token_tiles * n_chunks] fp32 partial sumexp.

Host (cheap, exact fp32):
  - math-token gather + segment-routed LoRA (A @ (B @ h) + bias): 2 GFLOP.
  - label logits   logit[t, label[t]] = tokens[t] . W[:, label[t]].
  - combine: lse = log(sum of 8 cores' sumexp); nll = lse - label_logit;
    masked means / per-segment means -> the 5 outputs.
"""

import sys

if "/opt/trn_rl_repo" not in sys.path:
    sys.path.insert(0, "/opt/trn_rl_repo")

import numpy as np
import ml_dtypes

BF16 = ml_dtypes.bfloat16

# problem shapes (hardcoded per spec)
B, S, H, V, Sm = 2, 1024, 4096, 32000, 256
NSEG, CHUNK = 16, 16
BETA1, BETA2, BETA3 = 0.5, 0.5, 0.4
N_CORES = 8
TOK = B * S + B * Sm            # 2560 tokens through the lm head
VS = V // N_CORES               # 4000 vocab columns per core

# device tiling
P = 128                         # partitions / token-tile size
CW = 500                        # vocab chunk width (<=512 fp32 PSUM bank)
NCC = VS // CW                  # 8 vocab chunks per core
KT = H // P                     # 32 K-tiles
N_PASS = 2                      # token passes (SBUF residency)
TT = TOK // (N_PASS * P)        # 10 token-tiles per pass

_cache = {}

# fp8 mode constants
F8 = ml_dtypes.float8_e4m3fn
W_SCALE = 64.0
KT2 = H // 256                  # 16 K-supertiles (256 rows each, DoubleRow)
CW8 = 512                       # fp8 vocab chunk width
VS8 = NCC * CW8                 # 4096 padded shard width (96 zero pads)
TT8 = TOK // P                  # 20 token tiles, single pass


def _build_nc_fp8(tt, kt2, ncc, cw, repeat=1, repeat_compute=1):
    """fp8-e4m3 DoubleRow variant: single pass, K=256 per matmul.

    Token count = tt*128, hidden = kt2*256, vocab shard = ncc*cw (incl
    zero padding). W is pre-scaled by 64 on the host; the Exp activation
    applies scale=1/64. Each pad column contributes exp(0)=1 to the
    sumexp; the host subtracts the pad count.
    """
    import concourse.mybir as mybir
    import concourse.tile as tile
    from concourse import bacc

    nc = bacc.Bacc("TRN2", target_bir_lowering=False, debug=False,
                   num_devices=N_CORES)
    f8 = mybir.dt.float8e4
    f32 = mybir.dt.float32

    KG = 4 if kt2 % 4 == 0 else kt2   # k2-super-tiles per hid DMA
    n_hg = kt2 // KG
    hid_d = nc.dram_tensor("hid", [n_hg, P, KG * 2 * tt * P], f8,
                           kind="ExternalInput")
    w_d = nc.dram_tensor("w", [ncc, P, kt2 * 2 * cw], f8,
                         kind="ExternalInput")
    se_d = nc.dram_tensor("se", [P, tt * ncc], f32, kind="ExternalOutput")

    with tile.TileContext(nc) as tc:
        with (
            tc.tile_pool(name="hidp", bufs=n_hg) as hidp,
            tc.tile_pool(name="wp", bufs=2) as wp,
            tc.tile_pool(name="sep", bufs=1) as sep,
            tc.tile_pool(name="exp", bufs=4) as expp,
            tc.tile_pool(name="ps", bufs=8, space="PSUM") as psp,
        ):
            se_all = sep.tile([P, tt * ncc], f32)
            for _rep in range(repeat):
                hid_t = []               # hid_t[g] : [P, KG, 2, tt*P]
                for g in range(n_hg):
                    t = hidp.tile([P, KG, 2, tt * P], f8, tag="hid")
                    nc.sync.dma_start(t[:], hid_d[g])
                    hid_t.append(t)
                for cc in range(ncc):
                    w_t = wp.tile([P, kt2, 2, cw], f8, tag="w")
                    nc.sync.dma_start(w_t[:], w_d[cc])
                    for _crep in range(repeat_compute):
                      for ti in range(tt):
                        ps = psp.tile([P, cw], f32, tag="ps")
                        for k in range(kt2):
                            nc.tensor.matmul(
                                ps[:],
                                hid_t[k // KG][:, k % KG, :,
                                               ti * P:(ti + 1) * P],
                                w_t[:, k, :, :],
                                start=(k == 0),
                                stop=(k == kt2 - 1),
                                perf_mode=mybir.MatmulPerfMode.DoubleRow,
                            )
                        ex = expp.tile([P, cw], f32, tag="ex")
                        col = ti * ncc + cc
                        nc.scalar.activation(
                            ex[:], ps[:],
                            mybir.ActivationFunctionType.Exp,
                            scale=1.0 / W_SCALE,
                            accum_out=se_all[:, col:col + 1],
                        )
            nc.sync.dma_start(se_d[:], se_all[:])
    nc.compile()
    return nc


def _patch_ldw_opt():
    """Flip walrus --enable-ldw-opt to true (dedupes LDWEIGHTS for
    consecutive matmuls that share a stationary operand; DoubleRow is
    LDWEIGHTS-bound since it disables FWL)."""
    if _cache.get("ldw_patched"):
        return
    import concourse.bass_utils as bu
    orig = bu.run_command

    def patched(argv, **kw):
        argv = ["--enable-ldw-opt=true" if a == "--enable-ldw-opt=false"
                else a for a in argv]
        return orig(argv, **kw)

    bu.run_command = patched
    _cache["ldw_patched"] = True


def _build_nc_fp8b(tt, kt2, ncc, cw, pair=2):
    """fp8 DoubleRow, cc-paired: for each (token-tile, k-step) issue
    `pair` matmuls that share the same stationary operand, so walrus
    ldw-opt can elide the redundant 256-column weight loads."""
    import concourse.mybir as mybir
    import concourse.tile as tile
    from concourse import bacc

    _patch_ldw_opt()
    nc = bacc.Bacc("TRN2", target_bir_lowering=False, debug=False,
                   num_devices=N_CORES)
    f8 = mybir.dt.float8e4
    f32 = mybir.dt.float32

    KG = 4 if kt2 % 4 == 0 else kt2
    n_hg = kt2 // KG
    hid_d = nc.dram_tensor("hid", [n_hg, P, KG * 2 * tt * P], f8,
                           kind="ExternalInput")
    w_d = nc.dram_tensor("w", [ncc, P, kt2 * 2 * cw], f8,
                         kind="ExternalInput")
    se_d = nc.dram_tensor("se", [P, tt * ncc], f32, kind="ExternalOutput")

    with tile.TileContext(nc) as tc:
        with (
            tc.tile_pool(name="hidp", bufs=n_hg) as hidp,
            tc.tile_pool(name="wp", bufs=2 * pair) as wp,
            tc.tile_pool(name="sep", bufs=1) as sep,
            tc.tile_pool(name="exp", bufs=4) as expp,
            tc.tile_pool(name="ps", bufs=8, space="PSUM") as psp,
        ):
            se_all = sep.tile([P, tt * ncc], f32)
            hid_t = []
            for g in range(n_hg):
                t = hidp.tile([P, KG, 2, tt * P], f8, tag="hid")
                nc.sync.dma_start(t[:], hid_d[g])
                hid_t.append(t)
            for ccg in range(ncc // pair):
                w_ts = []
                for j in range(pair):
                    w_t = wp.tile([P, kt2, 2, cw], f8, tag="w")
                    nc.sync.dma_start(w_t[:], w_d[ccg * pair + j])
                    w_ts.append(w_t)
                for ti in range(tt):
                    pss = [psp.tile([P, cw], f32, tag="ps", name=f"ps{j}")
                           for j in range(pair)]
                    for k in range(kt2):
                        lhs = hid_t[k // KG][:, k % KG, :,
                                             ti * P:(ti + 1) * P]
                        for j in range(pair):
                            nc.tensor.matmul(
                                pss[j][:], lhs, w_ts[j][:, k, :, :],
                                start=(k == 0), stop=(k == kt2 - 1),
                                perf_mode=mybir.MatmulPerfMode.DoubleRow,
                            )
                    for j in range(pair):
                        ex = expp.tile([P, cw], f32, tag="ex")
                        col = ti * ncc + ccg * pair + j
                        nc.scalar.activation(
                            ex[:], pss[j][:],
                            mybir.ActivationFunctionType.Exp,
                            scale=1.0 / W_SCALE,
                            accum_out=se_all[:, col:col + 1],
                        )
            nc.sync.dma_start(se_d[:], se_all[:])
    nc.compile()
    return nc


def _build_nc(n_pass, tt, kt, ncc, cw, hid_bufs, w_bufs, repeat=1,
              repeat_compute=1):
    """Build + compile the SPMD Bass kernel. Token count = n_pass*tt*128,
    hidden = kt*128, vocab shard = ncc*cw. repeat>1 duplicates the whole
    compute; repeat_compute>1 duplicates only the matmul+exp stage
    (timing-slope measurements only)."""
    import concourse.mybir as mybir
    import concourse.tile as tile
    from concourse import bacc

    nc = bacc.Bacc("TRN2", target_bir_lowering=False, debug=False,
                   num_devices=N_CORES)
    bf = mybir.dt.bfloat16
    f32 = mybir.dt.float32

    KG = 8 if kt % 8 == 0 else kt   # hid DMA granularity (k-tiles per DMA)
    n_hg = kt // KG
    hid_d = nc.dram_tensor("hid", [n_pass, n_hg, P, KG * tt * P], bf,
                           kind="ExternalInput")
    w_d = nc.dram_tensor("w", [ncc, P, kt * cw], bf, kind="ExternalInput")
    se_d = nc.dram_tensor("se", [P, n_pass * tt * ncc], f32,
                          kind="ExternalOutput")

    with tile.TileContext(nc) as tc:
        with (
            tc.tile_pool(name="hidp", bufs=hid_bufs) as hidp,
            tc.tile_pool(name="wp", bufs=w_bufs) as wp,
            tc.tile_pool(name="sep", bufs=1) as sep,
            tc.tile_pool(name="exp", bufs=4) as expp,
            tc.tile_pool(name="ps", bufs=8, space="PSUM") as psp,
        ):
            se_all = sep.tile([P, n_pass * tt * ncc], f32)
            for _rep in range(repeat):
              for pas in range(n_pass):
                hid_t = []                       # hid_t[g] : [P, KG, tt*P]
                for g in range(n_hg):
                    t = hidp.tile([P, KG, tt * P], bf, tag="hid")
                    nc.sync.dma_start(t[:], hid_d[pas, g])
                    hid_t.append(t)
                for cc in range(ncc):
                    w_t = wp.tile([P, kt, cw], bf, tag="w")
                    nc.sync.dma_start(w_t[:], w_d[cc])
                    for _crep in range(repeat_compute):
                      for ti in range(tt):
                        ps = psp.tile([P, cw], f32, tag="ps")
                        for k in range(kt):
                            nc.tensor.matmul(
                                ps[:],
                                hid_t[k // KG][:, k % KG,
                                               ti * P:(ti + 1) * P],
                                w_t[:, k, :],
                                start=(k == 0),
                                stop=(k == kt - 1),
                            )
                        ex = expp.tile([P, cw], f32, tag="ex")
                        col = (pas * tt + ti) * ncc + cc
                        nc.scalar.activation(
                            ex[:], ps[:],
                            mybir.ActivationFunctionType.Exp,
                            accum_out=se_all[:, col:col + 1],
                        )
            nc.sync.dma_start(se_d[:], se_all[:])
    nc.compile()
    return nc


MODE = "fp8"                    # "fp8" (DoubleRow) or "bf16"


def _get_nc():
    key = "full_" + MODE
    if key not in _cache:
        if MODE == "fp8":
            _cache[key] = _build_nc_fp8(TT8, KT2, NCC, CW8)
        else:
            _cache[key] = _build_nc(N_PASS, TT, KT, NCC, CW, hid_bufs=5,
                                    w_bufs=2)
    return _cache[key]


def _pack_inputs_fp8(tokens_f32, W_lm):
    """tokens [TOK, H] fp32, W_lm [H, V] fp32 -> per-core fp8 in_maps."""
    KG = 4
    n_hg = KT2 // KG
    # hid[g, p, (k2g, j, t)] = tokens[t, ((g*KG + k2g)*2 + j)*128 + p]
    hidT = tokens_f32.T.astype(F8)                                 # [H, TOK]
    hid = hidT.reshape(n_hg, KG, 2, P, TOK).transpose(0, 3, 1, 2, 4)
    hid = np.ascontiguousarray(hid).reshape(n_hg, P, KG * 2 * TOK)
    Ws = (W_lm * W_SCALE).astype(F8)                               # [H, V]
    in_maps = []
    for c in range(N_CORES):
        ws = Ws[:, c * VS:(c + 1) * VS]                            # [H, 4000]
        wp = np.zeros((H, VS8), F8)
        wp[:, :VS] = ws
        # w[cc, p, (k2, j, n)] = wp[(k2*2 + j)*128 + p, cc*cw + n]
        w = wp.reshape(KT2, 2, P, NCC, CW8).transpose(3, 2, 0, 1, 4)
        w = np.ascontiguousarray(w).reshape(NCC, P, KT2 * 2 * CW8)
        in_maps.append({"hid": hid, "w": w})
    return in_maps


def _pack_inputs(tokens_f32, W_lm):
    """tokens [TOK, H] fp32, W_lm [H, V] fp32 -> per-core in_maps."""
    KG = 8
    n_hg = KT // KG
    # hid[pas, g, p, (k8, t)] = tokens[pas*tt*128 + t, (g*KG + k8)*128 + p]
    hidT = tokens_f32.T.astype(BF16)                               # [H, TOK]
    hid = hidT.reshape(n_hg, KG, P, N_PASS, TT * P).transpose(3, 0, 2, 1, 4)
    hid = np.ascontiguousarray(hid).reshape(N_PASS, n_hg, P, KG * TT * P)
    W_b = W_lm.astype(BF16)                                        # [H, V]
    in_maps = []
    for c in range(N_CORES):
        ws = W_b[:, c * VS:(c + 1) * VS]                           # [H, 4000]
        # w[cc, p, (k, n)] = ws[k*128 + p, cc*cw + n]
        w = ws.reshape(KT, P, NCC, CW).transpose(2, 1, 0, 3)
        w = np.ascontiguousarray(w).reshape(NCC, P, KT * CW)
        in_maps.append({"hid": hid, "w": w})
    return in_maps


def _make_runner(nc):
    """Build a jitted SPMD executable for an already-compiled nc.

    Returns (fn, mesh, out_avals) where fn(hid_global, w_global,
    zero_se_global) -> [se_global]. hid is replicated; w/se are sharded
    on axis 0.  (Used by test.py for repeat-timing; kernel() itself goes
    through run_bass_kernel_spmd.)
    """
    import jax
    import concourse.mybir as mybir
    from concourse.bass2jax import (_bass_exec_p, install_neuronx_cc_hook,
                                    partition_id_tensor)
    from jax.experimental.shard_map import shard_map
    from jax.sharding import Mesh, PartitionSpec

    install_neuronx_cc_hook()

    part_name = (nc.partition_id_tensor.name
                 if nc.partition_id_tensor is not None else None)
    in_names, out_names, out_avals = [], [], []
    for alloc in nc.m.functions[0].allocations:
        if not isinstance(alloc, mybir.MemoryLocationSet):
            continue
        name = alloc.memorylocations[0].name
        if alloc.kind == "ExternalInput":
            if name != part_name:
                in_names.append(name)
        elif alloc.kind == "ExternalOutput":
            out_names.append(name)
            out_avals.append(jax.core.ShapedArray(
                tuple(alloc.tensor_shape), mybir.dt.np(alloc.dtype)))
    n_params = len(in_names)
    all_names = in_names + out_names
    if part_name is not None:
        all_names = all_names + [part_name]

    def _body(*args):
        operands = list(args)
        if part_name is not None:
            operands.append(partition_id_tensor())
        outs = _bass_exec_p.bind(
            *operands,
            out_avals=tuple(out_avals),
            in_names=tuple(all_names),
            out_names=tuple(out_names),
            lowering_input_output_aliases=(),
            sim_require_finite=True,
            sim_require_nnan=True,
            nc=nc,
        )
        return tuple(outs)

    devices = jax.devices()[:N_CORES]
    mesh = Mesh(np.asarray(devices), ("core",))
    # hid replicated, w + zero-out sharded along axis 0.  No donation:
    # the kernel writes every element of its outputs, so the pre-zeroed
    # "output" operands can be reused across calls (repeat timing).
    in_specs = (PartitionSpec(), PartitionSpec("core"), PartitionSpec("core"))
    out_specs = (PartitionSpec("core"),) * len(out_names)
    del n_params
    fn = jax.jit(
        shard_map(_body, mesh=mesh, in_specs=in_specs, out_specs=out_specs,
                  check_rep=False),
        keep_unused=True)
    return fn, mesh, out_avals


def _get_runner():
    if "runner" not in _cache:
        _cache["runner"] = _make_runner(_get_nc())
    return _cache["runner"]


def _run_device(tokens_f32, W_lm):
    """Returns sumexp [TOK] fp32 (full-vocab sum of exp(logits))."""
    from concourse.bass_utils import run_bass_kernel_spmd
    nc = _get_nc()
    if MODE == "fp8":
        in_maps = _pack_inputs_fp8(tokens_f32, W_lm)
    else:
        in_maps = _pack_inputs(tokens_f32, W_lm)
    res = run_bass_kernel_spmd(nc, in_maps, core_ids=list(range(N_CORES)))
    # se [128, n_tiles*ncc]; token t = tcol*128 + p, tcol = col // ncc
    n_tiles = TOK // P
    total = np.zeros((P, n_tiles), np.float64)
    for c in range(N_CORES):
        se = res.results[c]["se"].astype(np.float64)
        total += se.reshape(P, n_tiles, NCC).sum(axis=2)           # [128, 20]
    if MODE == "fp8":
        # each core's shard has VS8-VS zero-pad columns contributing exp(0)=1
        total -= N_CORES * (VS8 - VS)
    return total.T.reshape(-1)                                     # token-major


def _host_prep(inputs):
    """LoRA math tokens + token concat, all fp32 exact."""
    kh = np.asarray(inputs["k_hidden"], np.float32)
    lh = np.asarray(inputs["last_hidden"], np.float32)
    starts = np.asarray(inputs["starts"], np.int64)
    A = np.asarray(inputs["A_matrices"], np.float32)
    Bm = np.asarray(inputs["B_matrices"], np.float32)
    bias = np.asarray(inputs["bias"], np.float32)

    pos = starts[:, None] + np.arange(Sm)[None, :]                 # [B, Sm]
    math_h = np.take_along_axis(kh, pos[:, :, None], axis=1)       # [B, Sm, H]
    hb = math_h.reshape(B, NSEG, CHUNK, H)
    # grouped GEMM: [B*CHUNK, H] @ [H, R] per segment
    inter = np.einsum("bsch,srh->bscr", hb, Bm, optimize=True)
    trans = np.einsum("bscr,shr->bsch", inter, A, optimize=True)
    trans = trans + bias[None, :, None, :]
    t_math = np.ascontiguousarray(trans.reshape(B * Sm, H), dtype=np.float32)
    tokens = np.concatenate([lh.reshape(B * S, H), t_math], axis=0)
    return tokens


def kernel(**inputs):
    tokens = _host_prep(inputs)
    W = np.asarray(inputs["W_lm"], np.float32)
    input_ids = np.asarray(inputs["input_ids"], np.int64)
    attention_mask = np.asarray(inputs["attention_mask"], np.int64)
    starts = np.asarray(inputs["starts"], np.int64)
    ends = np.asarray(inputs["ends"], np.int64)
    mlabels = np.asarray(inputs["math_labels"], np.int64)

    sumexp = _run_device(tokens, W)                                # [2560]
    lse = np.log(sumexp)                                           # float64

    # exact label logits on host
    labels = input_ids[:, 1:]                                      # [B, S-1]
    Wg = W[:, labels.reshape(-1)]                                  # [H, B*(S-1)]
    tok_bs = tokens[:B * S].reshape(B, S, H)[:, :-1, :].reshape(-1, H)
    ll_simple = np.einsum("th,ht->t", tok_bs, Wg,
                          optimize=True).reshape(B, S - 1)
    Wm = W[:, mlabels.reshape(-1)]                                 # [H, B*Sm]
    ll_math = np.einsum("th,ht->t", tokens[B * S:], Wm, optimize=True)

    lse_bs = lse[:B * S].reshape(B, S)[:, :-1]
    nll = (lse_bs - ll_simple).astype(np.float32)                  # [B, S-1]
    mnll = (lse[B * S:] - ll_math).astype(np.float32)              # [B*Sm]

    idx = np.arange(S - 1)[None, :]
    real_len = attention_mask.sum(axis=1)[:, None]
    mask_simple = (idx >= starts[:, None] - 1) & (idx <= ends[:, None] - 1)
    mask_final = (idx >= ends[:, None]) & (idx < real_len - 1)

    def masked_mean(x, m):
        m = m.astype(np.float32)
        s = m.sum()
        if s > 0:
            return np.float32((x * m).sum() / max(s, np.float32(1.0)))
        return np.float32(0.0)

    simple_talk_loss = masked_mean(nll, mask_simple)
    final_answer_loss = masked_mean(nll, mask_final)
    math_loss = np.float32(mnll.mean())
    A_losses = mnll.reshape(B, NSEG, CHUNK).mean(axis=(0, 2)).astype(np.float32)
    total_loss = np.float32(BETA1 * math_loss + BETA2 * simple_talk_loss
                            + BETA3 * final_answer_loss)
    return (np.asarray(total_loss, np.float32),
            np.asarray(math_loss, np.float32),
            np.asarray(simple_talk_loss, np.float32),
            np.asarray(final_answer_loss, np.float32),
            A_losses)
